# revision 21
# baseline (speedup 1.0000x reference)
"""Trainium2 Bass kernel for multi-head attention (nn_Attention_54984171323822).

Reference computation (fp32):
    qkv = x @ w_qkv.T + b_qkv            # [B, N, 3*1024]
    q, k, v -> 16 heads x 64
    attn = softmax(q k^T / 8) v          # per head
    out = attn_flat @ w_out.T + b_out    # [B, N, 1024]

Shapes: B=4, N=2048, HIDDEN=1024, 16 heads x 64.

Sharding (8 NeuronCores): DP=4 over batch x TP=2 over heads. Core c handles
batch c//2 and heads (c%2)*8..(c%2)*8+8. No device collectives: each core
emits a partial output-projection [2048, 1024]; the host sums the TP pairs
and adds b_out (linear, so it commutes).

v4 phase-2 design:
  * Heads processed in PAIRS (hA at SBUF partitions 0:64, hB at 64:128 of one
    qkT chunk); EVERY phase-2 matmul runs in 64x64 PE-tiling mode, so the
    S^T, PV and denominator matmuls all pack four concurrent quadrants and
    the PE never pays a tiling-mode-switch drain inside the phase.
  * q is processed in 512-wide blocks; PV/den accumulate in four psum banks
    (P/P' for the two k-halves x both heads col-packed, D/D' likewise via
    ones-weight matmuls whose M=64 "junk broadcast" rows land the softmax
    denominator on exactly the partitions they normalize).  DVE merges
    P+P', recips D+D', and writes normalized bf16 attnT.
  * S^T psum tiles hold TWO k-chunks side by side, so exp() runs as one
    [128,1024] op per head per 2 chunks -- big enough to amortize engine
    overhead -- split across ScalarE (exact exp activation) and VectorE
    (Schraudolph bf16 bit-trick at ~1.5% RMS that washes out in the softmax
    average), alternating per chunk-pair to mix the error across k.

The no-max-subtraction softmax is safe here: logits are ~N(0, 0.5^2) after
the 1/8 scale, so exp() stays in (1e-3, ~20).
"""

import sys

sys.path.insert(0, "/opt/trn_rl_repo")

import numpy as np
import ml_dtypes

import concourse.bass as bass
import concourse.bacc as bacc
import concourse.tile as tile
from concourse import mybir
from concourse import bass_utils

N_CORES = 8
B = 4
N = 2048
HIDDEN = 1024
N_HEADS = 16
HEAD_DIM = 64
HPC = N_HEADS // 2          # heads per core (TP=2)
EC = HPC * HEAD_DIM         # 512 attention dims per core
NPAIR = HPC // 2            # 4 head pairs per core
TC = N // 128               # 16 token chunks
DC = HIDDEN // 128          # 8 hidden chunks
SCALE = HEAD_DIM ** -0.5

BF16 = mybir.dt.bfloat16
F32 = mybir.dt.float32
I16 = mybir.dt.int16
NP_BF16 = ml_dtypes.bfloat16

# Schraudolph exp in bf16 bits: exp(s/8) = 2^(s*log2e/8);
# bits16 = round(128 * (s*log2e/8) + 127*128 + c), c tuned for min RMS err.
EXP_A = 128.0 * SCALE * 1.4426950408889634
EXP_B = 16256.0 - 7.4


def _build_kernel_body(nc, tc_ctx, ios, dbg=None):
    import contextlib

    xT, wqkvT, bias_qk, bias_v, w_outT, out = ios
    tc = tc_ctx
    ctx = contextlib.ExitStack()
    with ctx:
        const = ctx.enter_context(tc.tile_pool(name="const", bufs=1))
        work = ctx.enter_context(tc.tile_pool(name="work", bufs=3))
        etp = ctx.enter_context(tc.tile_pool(name="etp", bufs=2))
        small = ctx.enter_context(tc.tile_pool(name="small", bufs=2))
        # PSUM budget (8 banks): stA + stB ([128,1024] f32, 2 banks each,
        # each holding TWO 512-q k-chunk score tiles side by side) +
        # pv0..pv3 ([128,512] f32: P/P'/D/D' in phase 2, rotation
        # accumulators in phases 1/3).
        psp = ctx.enter_context(tc.tile_pool(name="psp", bufs=1, space="PSUM"))

        _acc_i = [0]

        def acc_tile():
            t = psp.tile([128, 512], F32, tag=("pvP", "pvD")[_acc_i[0] % 2],
                         bufs=2, name="acc")
            _acc_i[0] += 1
            return t

        # ---- resident SBUF tensors ----
        xT_src = xT.ap().rearrange("(c p) t -> c p t", p=128)
        wq_src = wqkvT.ap().rearrange("(c p) e -> c p e", p=128)
        xT_c = []
        wq_c = []
        for dc in range(DC):
            wt = const.tile([128, 3 * EC], BF16, name=f"wq{dc}", tag=f"wq{dc}")
            nc.scalar.dma_start(out=wt[:], in_=wq_src[dc])
            wq_c.append(wt)
            xt = const.tile([128, N], BF16, name=f"xc{dc}", tag=f"xc{dc}")
            nc.sync.dma_start(out=xt[:], in_=xT_src[dc])
            xT_c.append(xt)
        wo_sb = const.tile([128, EC // 128, HIDDEN], BF16, name="wo_sb", tag="wo_sb")
        nc.sync.dma_start(out=wo_sb[:], in_=w_outT.ap().rearrange("(c p) e -> p c e", p=128))
        bqk_sb = const.tile([128, 8], F32, name="bqk_sb", tag="bqk_sb")
        nc.sync.dma_start(out=bqk_sb[:], in_=bias_qk.ap())
        bv_sb = const.tile([128, EC], BF16, name="bv_sb", tag="bv_sb")
        bv_ap = bias_v.ap()
        bv_bcast = bass.AP(tensor=bv_ap.tensor, offset=bv_ap.offset,
                           ap=[[0, 128], [1, EC]])
        nc.gpsimd.dma_start(out=bv_sb[:], in_=bv_bcast)

        qkT = const.tile([128, 2 * EC // 128, N], BF16, name="qkT", tag="qkT")   # [128, 8, 2048]
        vpp = const.tile([128, TC, EC], BF16, name="vpp", tag="vpp")             # [128, 16, 512]
        attnT_c = [const.tile([128, N], BF16, name=f"attnT{i}", tag=f"attnT{i}")
                   for i in range(EC // 128)]

        ones64 = const.tile([128, 64], BF16, name="ones64", tag="ones64")
        nc.vector.memset(ones64[:], 1.0)

        # ---- phase 1: qkT = w_qk @ x^T + b (e on partitions) ----
        for ec in range(2 * EC // 128):                      # 8 chunks (q then k)
            for ti in range(4):
                ps = acc_tile()
                for dc in range(DC):
                    # M=128: full-width weight loads engage the compiler's
                    # automatic Fast Weight Load (bf16, NumWeights==128)
                    nc.tensor.matmul(
                        ps[:],
                        wq_c[dc][:, ec * 128:(ec + 1) * 128],
                        xT_c[dc][:, ti * 512:(ti + 1) * 512],
                        start=(dc == 0), stop=(dc == DC - 1),
                    )
                nc.vector.tensor_scalar_add(
                    qkT[:, ec, ti * 512:(ti + 1) * 512], ps[:],
                    bqk_sb[:, ec:ec + 1],
                )

        # ---- phase 1b: V (tokens on partitions) + bias ----
        for ti in range(TC):
            ps = acc_tile()
            for dc in range(DC):
                nc.tensor.matmul(
                    ps[:],
                    xT_c[dc][:, ti * 128:(ti + 1) * 128],
                    wq_c[dc][:, 2 * EC:3 * EC],
                    start=(dc == 0), stop=(dc == DC - 1),
                )
            nc.vector.tensor_tensor(out=vpp[:, ti], in0=ps[:], in1=bv_sb[:],
                                    op=mybir.AluOpType.add)

        # ---- phase 2: attention, head pairs; q-block outer so the output
        # projection for each 512-token block interleaves right after its
        # four pair-blocks (the proj matmuls fill the exp-chain stalls) ----
        out3 = out.ap().rearrange("(t p) e -> t p e", p=128)
        for qq in range(4):                          # 512-wide q block
            for p in range(NPAIR):
                q0 = qq * 512
                # accumulators: P = pair-packed PV (hA dims at rows 0:64 via
                # col tile 0, hB at 64:128 via col tile 64); D = denominators
                # on the same partitions (M=64 "junk broadcast" rows).
                P = psp.tile([128, 512], F32, tag="pvP", bufs=2, name="P")
                D = psp.tile([128, 512], F32, tag="pvD", bufs=2, name="D")
                et_live = {}

                def emit_pv(kcp, P=P, D=D, et_live=et_live, p=p):
                    first, last = (kcp == 0), (kcp == TC - 1)
                    eA, eB = et_live[kcp // 2]
                    if kcp % 2 == 1:
                        del et_live[kcp // 2]
                    qs = slice((kcp % 2) * 512, (kcp % 2) * 512 + 512)
                    vc = slice(p * 128, p * 128 + 64)
                    vc2 = slice(p * 128 + 64, p * 128 + 128)
                    # (128, 64) col-tile pairs: the two P matmuls run
                    # concurrently, then the two D matmuls.
                    nc.tensor.matmul(
                        P[0:64, :], vpp[:, kcp, vc], eA[:, qs],
                        start=first, stop=last, skip_group_check=True)
                    nc.tensor.matmul(
                        P[64:128, :], vpp[:, kcp, vc2], eB[:, qs],
                        start=first, stop=last, skip_group_check=True)
                    nc.tensor.matmul(
                        D[0:64, :], ones64[:], eA[:, qs],
                        start=first, stop=last, skip_group_check=True)
                    nc.tensor.matmul(
                        D[64:128, :], ones64[:], eB[:, qs],
                        start=first, stop=last, skip_group_check=True)

                for kcb in range(TC // 2):           # 2-k-chunk batches
                    stA = psp.tile([128, 1024], F32, tag="stA", bufs=1,
                                   name="stA")
                    stB = psp.tile([128, 1024], F32, tag="stB", bufs=1,
                                   name="stB")
                    for j in range(2):
                        kc = 2 * kcb + j
                        for kh in range(2):
                            ks = slice(kc * 128 + kh * 64, kc * 128 + (kh + 1) * 64)
                            od = slice(kh * 64, (kh + 1) * 64)
                            os_ = slice(j * 512, (j + 1) * 512)
                            qq_s = slice(q0, q0 + 512)
                            nc.tensor.matmul(
                                stA[od, os_], qkT[0:64, 4 + p, ks],
                                qkT[0:64, p, qq_s], start=True, stop=True)
                            nc.tensor.matmul(
                                stB[od, os_], qkT[64:128, 4 + p, ks],
                                qkT[64:128, p, qq_s], start=True, stop=True)
                    etA = etp.tile([128, 1024], BF16, tag="etA", bufs=2, name="etA")
                    etB = etp.tile([128, 1024], BF16, tag="etB", bufs=2, name="etB")
                    if kcb % 2 == 0:
                        sct, dvt, scs, dvs = etA, etB, stA, stB
                    else:
                        sct, dvt, scs, dvs = etB, etA, stB, stA
                    nc.scalar.activation(
                        out=sct[:], in_=scs[:],
                        func=mybir.ActivationFunctionType.Exp, scale=SCALE)
                    nc.vector.tensor_scalar(
                        out=dvt[:].bitcast(I16), in0=dvs[:],
                        scalar1=EXP_A, scalar2=EXP_B,
                        op0=mybir.AluOpType.mult, op1=mybir.AluOpType.add)
                    et_live[kcb] = (etA, etB)
                    if kcb > 0:
                        emit_pv(2 * kcb - 2)
                        emit_pv(2 * kcb - 1)
                emit_pv(TC - 2)
                emit_pv(TC - 1)

                # normalize: attnT = P * recip(D) -- denominators already sit
                # on the partitions of the V dims they normalize.
                rec = small.tile([128, 512], F32, tag="rec", bufs=2, name="rec")
                nc.vector.reciprocal_approx_fast(out=rec[:], in_=D[:])
                nc.vector.tensor_tensor(
                    out=attnT_c[p][:, q0:q0 + 512],
                    in0=P[:], in1=rec[:], op=mybir.AluOpType.mult)

            # ---- projection for this q block's 4 token chunks ----
            for tiq in range(4):
                ti = qq * 4 + tiq
                osb = work.tile([128, HIDDEN], F32, name="osb", tag="osb")
                for e5 in range(2):
                    po = acc_tile()
                    for acx in range(EC // 128):
                        nc.tensor.matmul(
                            po[:],
                            attnT_c[acx][:, ti * 128:(ti + 1) * 128],
                            wo_sb[:, acx, e5 * 512:(e5 + 1) * 512],
                            start=(acx == 0), stop=(acx == EC // 128 - 1),
                        )
                    nc.scalar.copy(osb[:, e5 * 512:(e5 + 1) * 512], po[:])
                nc.sync.dma_start(out=out3[ti], in_=osb[:])

        if dbg is not None:
            for nm, t in (("qkT", qkT), ("vpp", vpp)):
                if nm in dbg:
                    nc.sync.dma_start(out=dbg[nm].ap(), in_=t[:])
            if "attnT" in dbg:
                for i in range(4):
                    nc.sync.dma_start(out=dbg["attnT"].ap()[:, i], in_=attnT_c[i][:])


def build_nc(debug_dump=False, num_devices=N_CORES):
    nc = bacc.Bacc("TRN2", target_bir_lowering=False, debug=False,
                   num_devices=num_devices)
    xT = nc.dram_tensor("xT", [HIDDEN, N], BF16, kind="ExternalInput")
    wqkvT = nc.dram_tensor("wqkvT", [HIDDEN, 3 * EC], BF16, kind="ExternalInput")
    bias_qk = nc.dram_tensor("bias_qk", [128, 8], F32, kind="ExternalInput")
    bias_v = nc.dram_tensor("bias_v", [1, EC], BF16, kind="ExternalInput")
    w_outT = nc.dram_tensor("w_outT", [EC, HIDDEN], BF16, kind="ExternalInput")
    out = nc.dram_tensor("out", [N, HIDDEN], F32, kind="ExternalOutput")
    dbg = None
    if debug_dump:
        dbg = {
            "qkT": nc.dram_tensor("dbg_qkT", [128, 8, N], BF16, kind="ExternalOutput"),
            "vpp": nc.dram_tensor("dbg_vpp", [128, TC, EC], BF16, kind="ExternalOutput"),
            "attnT": nc.dram_tensor("dbg_attnT", [128, 4, N], BF16, kind="ExternalOutput"),
        }
    with tile.TileContext(nc) as tc:
        _build_kernel_body(nc, tc, (xT, wqkvT, bias_qk, bias_v, w_outT, out), dbg=dbg)
    nc.compile()
    return nc


def make_in_maps(x, w_qkv, b_qkv, w_out):
    """Shard the full inputs into 8 per-core input maps."""
    in_maps = []
    for c in range(N_CORES):
        b = c // 2
        tp = c % 2
        sl = slice(tp * EC, (tp + 1) * EC)
        xT_c = np.ascontiguousarray(x[b].T).astype(NP_BF16)
        wq = w_qkv[sl, :]
        wk = w_qkv[HIDDEN + tp * EC: HIDDEN + (tp + 1) * EC, :]
        wv = w_qkv[2 * HIDDEN + tp * EC: 2 * HIDDEN + (tp + 1) * EC, :]
        wqkvT_c = np.concatenate([wq, wk, wv], axis=0).T.astype(NP_BF16)
        wqkvT_c = np.ascontiguousarray(wqkvT_c)
        bq = b_qkv[tp * EC:(tp + 1) * EC]
        bk = b_qkv[HIDDEN + tp * EC: HIDDEN + (tp + 1) * EC]
        bv = b_qkv[2 * HIDDEN + tp * EC: 2 * HIDDEN + (tp + 1) * EC]
        bias_qk_c = np.concatenate([bq, bk]).reshape(8, 128).T.astype(np.float32)
        bias_qk_c = np.ascontiguousarray(bias_qk_c)
        bias_v_c = np.ascontiguousarray(bv.reshape(1, EC)).astype(NP_BF16)
        w_outT_c = np.ascontiguousarray(w_out[:, sl].T).astype(NP_BF16)
        in_maps.append({
            "xT": xT_c,
            "wqkvT": wqkvT_c,
            "bias_qk": bias_qk_c,
            "bias_v": bias_v_c,
            "w_outT": w_outT_c,
        })
    return in_maps


def combine_outputs(results, b_out):
    """results: list of 8 per-core {'out': [N, HIDDEN]} -> full [B, N, HIDDEN]."""
    out = np.empty((B, N, HIDDEN), np.float32)
    for b in range(B):
        out[b] = results[2 * b]["out"] + results[2 * b + 1]["out"]
        out[b] += b_out[None, :].astype(np.float32)
    return out


_NC = None


def _get_nc():
    global _NC
    if _NC is None:
        _NC = build_nc()
    return _NC


def kernel(x, w_qkv, b_qkv, w_out, b_out):
    x = np.asarray(x, np.float32)
    w_qkv = np.asarray(w_qkv, np.float32)
    b_qkv = np.asarray(b_qkv, np.float32)
    w_out = np.asarray(w_out, np.float32)
    b_out = np.asarray(b_out, np.float32)
    nc = _get_nc()
    in_maps = make_in_maps(x, w_qkv, b_qkv, w_out)
    res = bass_utils.run_bass_kernel_spmd(nc, in_maps, core_ids=list(range(N_CORES)))
    return combine_outputs(res.results, b_out)
